# revision 5
# baseline (speedup 1.0000x reference)
"""Masked multi-head attention on 8 trn2 NeuronCores (Bass/Tile).

B=1, N=4096, C=256, H=8 (Dh=32); dense 0/1 mask shared across heads.
Sequence-parallel over query rows: core i handles query rows
[512*i, 512*(i+1)) for all heads; k/v recomputed per core from the full
x; the mask is split 8 ways (no duplication), no collectives.

v2 schedule: steps alternate A (2 ktiles x 2 heads -> one 2048-wide exp
from a 4-bank PSUM tile) and B (1 ktile x 2 heads -> 1024-wide exp from
a 2-bank tile); pv matmuls lag one step (software pipeline) so the PE
never waits on the exp/mask chain.  PSUM tags: sA(1x4 banks) +
sB(1x2) + pv(2x1) = 8 banks; phase-1 q/k/v emissions borrow ring slots.
v-copies ride the idle GpSimd engine; the output projection accumulates
per-pair into an SBUF f32 tile (DVE adds) so the tail is ~5us.
"""

import sys

for _p in ("/opt/trn_rl_repo", "/root/.axon_site/_ro/trn_rl_repo"):
    if _p not in sys.path:
        sys.path.insert(0, _p)

import numpy as np
import ml_dtypes

BF16NP = ml_dtypes.bfloat16

N = 4096
C = 256
H = 8
DH = 32
NCORES = 8
NQ = N // NCORES  # 512 query rows per core
KT = N // 128  # 32 key tiles

_CACHE = {}


def build_kernel():
    import concourse.bacc as bacc
    import concourse.tile as tile
    from concourse import mybir
    import concourse.bass as bass

    F32 = mybir.dt.float32
    BF = mybir.dt.bfloat16
    EXP = mybir.ActivationFunctionType.Exp
    ADD = mybir.AluOpType.add

    nc = bacc.Bacc("TRN2", target_bir_lowering=False, debug=False, num_devices=NCORES)

    xT_d = nc.dram_tensor("xT", [C, N], BF, kind="ExternalInput")
    xqT_d = nc.dram_tensor("xqT", [C, NQ], BF, kind="ExternalInput")
    wqkv_d = nc.dram_tensor("wqkv", [C, 3 * C], BF, kind="ExternalInput")
    wproj2_d = nc.dram_tensor("wproj2", [4 * 128, C], BF, kind="ExternalInput")
    bias2_d = nc.dram_tensor("bias2", [128, 2], F32, kind="ExternalInput")
    maskT_d = nc.dram_tensor("maskT", [N, NQ], BF, kind="ExternalInput")
    out_d = nc.dram_tensor("out", [C, NQ], F32, kind="ExternalOutput")

    TAGBUFS = {"sA": 1, "sB": 1, "pv": 2}
    TAGCOLS = {"sA": 2048, "sB": 1024, "pv": 512}

    with (
        tile.TileContext(nc) as tc,
        tc.tile_pool(name="consts", bufs=1) as consts,
        tc.tile_pool(name="ps", bufs=1, space="PSUM") as ps,
        tc.tile_pool(name="pp", bufs=2) as pp,
        tc.tile_pool(name="dramp", bufs=1, space="DRAM") as dram_pool,
    ):
        # ---------------- input DMAs ----------------
        # sync queue: the head-phase critical path (HWDGE, low latency)
        w_sb = [
            consts.tile([128, 3 * C], BF, name=f"w_sb{c}", tag=f"w{c}")
            for c in range(2)
        ]
        for c in range(2):
            nc.sync.dma_start(out=w_sb[c], in_=wqkv_d[128 * c : 128 * (c + 1), :])
        xq_sb = [
            consts.tile([128, NQ], BF, name=f"xq_sb{c}", tag=f"xq{c}") for c in range(2)
        ]
        for c in range(2):
            nc.sync.dma_start(out=xq_sb[c], in_=xqT_d[128 * c : 128 * (c + 1), :])
        xT_sb = [
            consts.tile([128, N], BF, name=f"xT_sb{c}", tag=f"xT{c}") for c in range(2)
        ]
        for c in range(2):
            nc.sync.dma_start(
                out=xT_sb[c][:, 0:512], in_=xT_d[128 * c : 128 * (c + 1), 0:512]
            )
        for c in range(2):
            nc.sync.dma_start(
                out=xT_sb[c][:, 512:2048], in_=xT_d[128 * c : 128 * (c + 1), 512:2048]
            )
        # gpsimd queue: mask + bulk tail (SWDGE)
        mask_sb = consts.tile([128, KT, NQ], BF, name="mask_sb", tag="mask")
        maskT_r = maskT_d.rearrange("(m p) q -> p m q", p=128)
        nc.gpsimd.dma_start(out=mask_sb[:, 0:4, :], in_=maskT_r[:, 0:4, :])
        for c in range(2):
            nc.gpsimd.dma_start(
                out=xT_sb[c][:, 2048:3072], in_=xT_d[128 * c : 128 * (c + 1), 2048:3072]
            )
        for c in range(2):
            nc.gpsimd.dma_start(
                out=xT_sb[c][:, 3072:4096], in_=xT_d[128 * c : 128 * (c + 1), 3072:4096]
            )
        for ch in range(1, 8):
            nc.gpsimd.dma_start(
                out=mask_sb[:, 4 * ch : 4 * (ch + 1), :],
                in_=maskT_r[:, 4 * ch : 4 * (ch + 1), :],
            )
        wp_sb = consts.tile([128, 4, C], BF, name="wp_sb", tag="wp")
        nc.gpsimd.dma_start(
            out=wp_sb, in_=wproj2_d.rearrange("(g p) c -> p g c", p=128)
        )
        bias_sb = consts.tile([128, 2], F32, name="bias_sb", tag="bias")
        nc.gpsimd.dma_start(out=bias_sb, in_=bias2_d[:])

        # ---------------- persistent SBUF ----------------
        qT_sb = [
            consts.tile([128, NQ], BF, name=f"qT_sb{g}", tag=f"qT{g}") for g in range(2)
        ]
        kT_sb = [
            consts.tile([128, N], BF, name=f"kT_sb{g}", tag=f"kT{g}") for g in range(2)
        ]
        # v tiles with a fused ones column: per ktile, 34-wide blocks
        # [v_h (32) | 1 | pad] so lhsT [128, 33] per head fuses the softmax
        # denominator into the pv matmul as output row 32.
        v_all = consts.tile([128, KT, 34 * H], BF, name="v_all", tag="vall")
        rec_sb = consts.tile([128, 4 * NQ], F32, name="rec_sb", tag="rec")
        bc_cat = consts.tile([128, 4 * NQ], F32, name="bc_cat", tag="bc")
        o_cat = [
            consts.tile([128, NQ], BF, name=f"o_cat{j}", tag=f"oc{j}") for j in range(4)
        ]
        f_acc = consts.tile([128, 1024], F32, name="f_acc", tag="facc")
        nrm_scr = dram_pool.tile([4, 2, NQ], F32)

        nc.gpsimd.memset(bc_cat, 0.0)
        v_r = v_all.rearrange("p m (h w) -> p m h w", h=H)
        for h in range(H):
            nc.gpsimd.memset(v_r[:, :, h, 32:34], 1.0)

        # ---------------- phase-1 emissions ----------------
        def emit_q(g, tag):
            q_ps = ps.tile(
                [128, TAGCOLS[tag]], F32, name="q_ps", tag=tag, bufs=TAGBUFS[tag]
            )
            for c in range(2):
                nc.tensor.matmul(
                    out=q_ps[:, 0:NQ],
                    lhsT=w_sb[c][:, 128 * g : 128 * (g + 1)],
                    rhs=xq_sb[c],
                    start=(c == 0),
                    stop=(c == 1),
                )
            nc.vector.tensor_copy(out=qT_sb[g], in_=q_ps[:, 0:NQ])

        def emit_kT(g, n, tag):
            k_ps = ps.tile(
                [128, TAGCOLS[tag]], F32, name="k_ps", tag=tag, bufs=TAGBUFS[tag]
            )
            for c in range(2):
                nc.tensor.matmul(
                    out=k_ps[:, 0:512],
                    lhsT=w_sb[c][:, 256 + 128 * g : 256 + 128 * (g + 1)],
                    rhs=xT_sb[c][:, 512 * n : 512 * (n + 1)],
                    start=(c == 0),
                    stop=(c == 1),
                )
            nc.vector.tensor_copy(
                out=kT_sb[g][:, 512 * n : 512 * (n + 1)], in_=k_ps[:, 0:512]
            )

        def emit_v(m, tag):
            v_ps = ps.tile(
                [128, TAGCOLS[tag]], F32, name="v_ps", tag=tag, bufs=TAGBUFS[tag]
            )
            for c in range(2):
                nc.tensor.matmul(
                    out=v_ps[:, 0:C],
                    lhsT=xT_sb[c][:, 128 * m : 128 * (m + 1)],
                    rhs=w_sb[c][:, 512:768],
                    start=(c == 0),
                    stop=(c == 1),
                )
            nc.vector.tensor_copy(
                out=v_r[:, m, :, 0:32],
                in_=v_ps[:, 0:C].rearrange("p (h w) -> p h w", h=H),
            )

        # head: q(g0) + v(0..19) + kT(0, 0..3), spread over the psum rings
        head_ems = [lambda t: emit_q(0, t)]
        head_ems += [(lambda m: lambda t: emit_v(m, t))(m) for m in range(20)]
        head_ems += [(lambda n: lambda t: emit_kT(0, n, t))(n) for n in range(4)]
        ring = ["sA", "sB", "pv", "pv"]
        for i, em in enumerate(head_ems):
            em(ring[i % 4])

        # per-pair emission queues (popped one per step, alternating rings)
        emq = {j: [] for j in range(4)}
        emq[0] = [
            (lambda m: lambda t: emit_v(m, t))(20),
            (lambda m: lambda t: emit_v(m, t))(21),
            (lambda n: lambda t: emit_kT(0, n, t))(4),
            (lambda m: lambda t: emit_v(m, t))(22),
            (lambda m: lambda t: emit_v(m, t))(23),
            (lambda n: lambda t: emit_kT(0, n, t))(5),
            (lambda m: lambda t: emit_v(m, t))(24),
            (lambda m: lambda t: emit_v(m, t))(25),
            (lambda n: lambda t: emit_kT(0, n, t))(6),
            (lambda m: lambda t: emit_v(m, t))(26),
            (lambda m: lambda t: emit_v(m, t))(27),
            (lambda n: lambda t: emit_kT(0, n, t))(7),
            (lambda m: lambda t: emit_v(m, t))(28),
            (lambda m: lambda t: emit_v(m, t))(29),
            (lambda m: lambda t: emit_v(m, t))(30),
            (lambda m: lambda t: emit_v(m, t))(31),
        ]
        emq[1] = [(lambda n: lambda t: emit_kT(1, n, t))(n) for n in range(8)]
        emq[1].append(lambda t: emit_q(1, t))

        # ---------------- phase 2: attention ----------------
        pv_tiles = {}

        def get_pv(j):
            if j not in pv_tiles:
                t = ps.tile([128, 512], F32, name="pv_t", tag="pv", bufs=2)
                pv_tiles[j] = t
                if j < 2:
                    # 1.0 keeps reciprocal_approx_fast well-defined on rows
                    # the pv matmuls never write; later pairs reuse the slot,
                    # whose unwritten rows still hold this memset.
                    nc.vector.memset(t, 1.0)
            return pv_tiles[j]

        def issue_pv(j, kind, m, p_t):
            hA, hB = 2 * j, 2 * j + 1
            pv_t = get_pv(j)
            ktcols = (
                [(m, 0), (m + 1, 512)] if kind == "A" else [(m, 0)]
            )
            for h, base in ((hA, 0), (hB, 1)):
                rows = (0, 33) if base == 0 else (64, 97)
                tp = (0, 0) if base == 0 else (0, 64)
                for kt, off in ktcols:
                    cof = off + (0 if base == 0 else (1024 if kind == "A" else 512))
                    nc.tensor.matmul(
                        out=pv_t[rows[0] : rows[1], 0:NQ],
                        lhsT=v_all[:, kt, 34 * h : 34 * h + 33],
                        rhs=p_t[:, cof : cof + NQ],
                        start=(kt == 0),
                        stop=(kt == KT - 1),
                        tile_position=tp,
                        skip_group_check=True,
                    )

        def epilogue(j):
            pv_t = pv_tiles.pop(j)
            nc.vector.reciprocal_approx_fast(
                out=rec_sb[:, NQ * j : NQ * j + NQ], in_=pv_t[:, 0:NQ]
            )
            for half, prow, orow in ((0, 32, 0), (1, 96, 64)):
                nc.sync.dma_start(
                    out=nrm_scr[j, half, :],
                    in_=rec_sb[prow : prow + 1, NQ * j : NQ * j + NQ],
                )
                row = nrm_scr[j, half : half + 1, :]
                bcast = bass.AP(
                    tensor=row.tensor,
                    offset=row.offset,
                    ap=[[0, 32]] + list(row.ap[1:]),
                )
                nc.sync.dma_start(
                    out=bc_cat[orow : orow + 32, NQ * j : NQ * j + NQ],
                    in_=bcast,
                )
            nc.vector.tensor_mul(
                out=o_cat[j],
                in0=pv_t[:, 0:NQ],
                in1=bc_cat[:, NQ * j : NQ * j + NQ],
            )
            f_ps = ps.tile([128, 1024], F32, name="f_ps", tag="sB", bufs=1)
            for t in range(2):
                nc.tensor.matmul(
                    out=f_ps[:, 512 * t : 512 * (t + 1)],
                    lhsT=wp_sb[:, j, 128 * t : 128 * (t + 1)],
                    rhs=o_cat[j],
                    start=True,
                    stop=True,
                )
            if j == 0:
                for t in range(2):
                    nc.vector.tensor_scalar(
                        out=f_acc[:, 512 * t : 512 * (t + 1)],
                        in0=f_ps[:, 512 * t : 512 * (t + 1)],
                        scalar1=bias_sb[:, t : t + 1],
                        scalar2=None,
                        op0=ADD,
                    )
            else:
                nc.vector.tensor_add(out=f_acc, in0=f_acc, in1=f_ps)

        # step list: per pair, (A: kt m,m+1 | B: kt m) pattern, 3 kt / cycle
        all_steps = []
        for j in range(4):
            for cyc in range(10):
                all_steps.append((j, "A", 3 * cyc))
                all_steps.append((j, "B", 3 * cyc + 2))
            all_steps.append((j, "A", 30))

        em_ring = {0: 0, 1: 0, 2: 0, 3: 0}
        pend = None
        for j, kind, m in all_steps:
            hA, hB = 2 * j, 2 * j + 1
            gA = hA // 4
            pA, pB = 32 * (hA % 4), 32 * (hB % 4)
            ncols = 2048 if kind == "A" else 1024
            s_t = ps.tile(
                [128, ncols], F32, name="s_t",
                tag=("sA" if kind == "A" else "sB"), bufs=1,
            )
            p_t = pp.tile(
                [128, ncols], BF, name="p_t",
                tag=("pA" if kind == "A" else "pB"), bufs=2,
            )
            kts = [m, m + 1] if kind == "A" else [m]
            # scores: out (kpos, qrow) per (head, ktile)
            for hi, (h, p_off) in enumerate(((hA, pA), (hB, pB))):
                for ki, kt in enumerate(kts):
                    cof = (hi * len(kts) + ki) * 512
                    nc.tensor.matmul(
                        out=s_t[:, cof : cof + NQ],
                        lhsT=kT_sb[gA][p_off : p_off + 32, 128 * kt : 128 * (kt + 1)],
                        rhs=qT_sb[gA][p_off : p_off + 32, :],
                        start=True,
                        stop=True,
                        tile_position=(p_off, 0),
                    )
            nc.scalar.activation(out=p_t, in_=s_t, func=EXP)
            if kind == "A":
                nc.vector.tensor_mul(
                    out=p_t[:, 0:1024], in0=p_t[:, 0:1024], in1=mask_sb[:, m : m + 2, :]
                )
                nc.vector.tensor_mul(
                    out=p_t[:, 1024:2048],
                    in0=p_t[:, 1024:2048],
                    in1=mask_sb[:, m : m + 2, :],
                )
            else:
                nc.vector.tensor_mul(
                    out=p_t[:, 0:512], in0=p_t[:, 0:512], in1=mask_sb[:, m, :]
                )
                nc.vector.tensor_mul(
                    out=p_t[:, 512:1024], in0=p_t[:, 512:1024], in1=mask_sb[:, m, :]
                )
            if pend is not None:
                pj, pkind, pm, pp_t = pend
                issue_pv(pj, pkind, pm, pp_t)
                if pkind == "A" and pm == 30:
                    epilogue(pj)
            if emq[j]:
                em = emq[j].pop(0)
                tag = ("sB", "sA")[em_ring[j] % 2]
                em_ring[j] += 1
                em(tag)
            pend = (j, kind, m, p_t)

        pj, pkind, pm, pp_t = pend
        issue_pv(pj, pkind, pm, pp_t)
        epilogue(pj)

        for t in range(2):
            nc.sync.dma_start(
                out=out_d[128 * t : 128 * (t + 1), :],
                in_=f_acc[:, 512 * t : 512 * (t + 1)],
            )

    nc.compile()
    return nc


def _get_nc():
    if "nc" not in _CACHE:
        _CACHE["nc"] = build_kernel()
    return _CACHE["nc"]


def _prep_inputs(x, adj, w_qkv, w_proj, b_proj):
    x = np.asarray(x, dtype=np.float32).reshape(N, C)
    adj = np.asarray(adj).reshape(N, N)
    w_qkv = np.asarray(w_qkv, dtype=np.float32)
    w_proj = np.asarray(w_proj, dtype=np.float32)
    b_proj = np.asarray(b_proj, dtype=np.float32)

    scale = float(DH) ** -0.5
    wqkvT = w_qkv.T.copy()
    wqkvT[:, 0:C] *= scale  # fold attention scale into q projection
    wqkvT = np.ascontiguousarray(wqkvT, dtype=BF16NP)
    wprojT = w_proj.T.astype(np.float32)  # [C (contraction), C (out)]
    # zero-padded reorder: block j rows 0:32 = head 2j, rows 64:96 =
    # head 2j+1, rest zero (matches pv bank partition layout)
    wproj2 = np.zeros((4 * 128, C), dtype=np.float32)
    for j in range(4):
        wproj2[128 * j + 0 : 128 * j + 32] = wprojT[64 * j : 64 * j + 32]
        wproj2[128 * j + 64 : 128 * j + 96] = wprojT[64 * j + 32 : 64 * j + 64]
    wproj2 = np.ascontiguousarray(wproj2, dtype=BF16NP)
    bias2 = np.ascontiguousarray(b_proj.reshape(2, 128).T, dtype=np.float32)
    xT = np.ascontiguousarray(x.T, dtype=BF16NP)
    adjT = (adj > 0).astype(BF16NP).T  # [kpos, qrow] 0/1

    in_maps = []
    for i in range(NCORES):
        sl = slice(NQ * i, NQ * (i + 1))
        in_maps.append(
            {
                "xT": xT,
                "xqT": np.ascontiguousarray(xT[:, sl]),
                "wqkv": wqkvT,
                "wproj2": wproj2,
                "bias2": bias2,
                "maskT": np.ascontiguousarray(adjT[:, sl]),
            }
        )
    return in_maps


def run_on_hw(inputs, trace=False):
    from concourse.bass_utils import run_bass_kernel_spmd

    if trace:
        import axon_profile_shim  # noqa: F401

    nc = _get_nc()
    in_maps = _prep_inputs(**inputs)
    res = run_bass_kernel_spmd(
        nc, in_maps, core_ids=list(range(NCORES)), trace=trace
    )
    out = np.empty((1, N, C), dtype=np.float32)
    for i in range(NCORES):
        out[0, NQ * i : NQ * (i + 1), :] = res.results[i]["out"].T
    return out, res


def kernel(x, adj, w_qkv, w_proj, b_proj):
    out, _ = run_on_hw(
        {"x": x, "adj": adj, "w_qkv": w_qkv, "w_proj": w_proj, "b_proj": b_proj}
    )
    return out


# revision 8
# speedup vs baseline: 1.2414x; 1.2414x over previous
"""Masked multi-head attention on 8 trn2 NeuronCores (Bass/Tile).

B=1, N=4096, C=256, H=8 (Dh=32); dense 0/1 mask shared across heads.
Sequence-parallel over query rows: core i handles query rows
[512*i, 512*(i+1)) for all heads; k/v recomputed per core from the full
x; the mask is split 8 ways (no duplication), no collectives.

v2 schedule: steps alternate A (2 ktiles x 2 heads -> one 2048-wide exp
from a 4-bank PSUM tile) and B (1 ktile x 2 heads -> 1024-wide exp from
a 2-bank tile); pv matmuls lag one step (software pipeline) so the PE
never waits on the exp/mask chain.  PSUM tags: sA(1x4 banks) +
sB(1x2) + pv(2x1) = 8 banks; phase-1 q/k/v emissions borrow ring slots.
v-copies ride the idle GpSimd engine; the output projection accumulates
per-pair into an SBUF f32 tile (DVE adds) so the tail is ~5us.
"""

import sys

for _p in ("/opt/trn_rl_repo", "/root/.axon_site/_ro/trn_rl_repo"):
    if _p not in sys.path:
        sys.path.insert(0, _p)

import numpy as np
import ml_dtypes

BF16NP = ml_dtypes.bfloat16

N = 4096
C = 256
H = 8
DH = 32
NCORES = 8
NQ = N // NCORES  # 512 query rows per core
KT = N // 128  # 32 key tiles

_CACHE = {}


def build_kernel():
    import concourse.bacc as bacc
    import concourse.tile as tile
    from concourse import mybir
    import concourse.bass as bass

    F32 = mybir.dt.float32
    BF = mybir.dt.bfloat16
    EXP = mybir.ActivationFunctionType.Exp
    ADD = mybir.AluOpType.add

    nc = bacc.Bacc("TRN2", target_bir_lowering=False, debug=False, num_devices=NCORES)

    xT_d = nc.dram_tensor("xT", [C, N], BF, kind="ExternalInput")
    xqT_d = nc.dram_tensor("xqT", [C, NQ], BF, kind="ExternalInput")
    wqkv_d = nc.dram_tensor("wqkv", [C, 3 * C], BF, kind="ExternalInput")
    wproj2_d = nc.dram_tensor("wproj2", [4 * 128, C], BF, kind="ExternalInput")
    bias2_d = nc.dram_tensor("bias2", [128, 2], F32, kind="ExternalInput")
    maskT_d = nc.dram_tensor("maskT", [N, NQ], BF, kind="ExternalInput")
    out_d = nc.dram_tensor("out", [C, NQ], F32, kind="ExternalOutput")

    TAGBUFS = {"sA": 1, "sB": 1, "pv": 2}
    TAGCOLS = {"sA": 2048, "sB": 1024, "pv": 512}

    with (
        tile.TileContext(nc) as tc,
        tc.tile_pool(name="consts", bufs=1) as consts,
        tc.tile_pool(name="ps", bufs=1, space="PSUM") as ps,
        tc.tile_pool(name="pp", bufs=2) as pp,
        tc.tile_pool(name="dramp", bufs=1, space="DRAM") as dram_pool,
    ):
        # ---------------- input DMAs ----------------
        # sync queue: the head-phase critical path (HWDGE, low latency)
        w_sb = [
            consts.tile([128, 3 * C], BF, name=f"w_sb{c}", tag=f"w{c}")
            for c in range(2)
        ]
        for c in range(2):
            nc.sync.dma_start(out=w_sb[c], in_=wqkv_d[128 * c : 128 * (c + 1), :])
        xq_sb = [
            consts.tile([128, NQ], BF, name=f"xq_sb{c}", tag=f"xq{c}") for c in range(2)
        ]
        for c in range(2):
            nc.sync.dma_start(out=xq_sb[c], in_=xqT_d[128 * c : 128 * (c + 1), :])
        xT_sb = [
            consts.tile([128, N], BF, name=f"xT_sb{c}", tag=f"xT{c}") for c in range(2)
        ]
        for c in range(2):
            nc.sync.dma_start(
                out=xT_sb[c][:, 0:512], in_=xT_d[128 * c : 128 * (c + 1), 0:512]
            )
        for c in range(2):
            nc.sync.dma_start(
                out=xT_sb[c][:, 512:2048], in_=xT_d[128 * c : 128 * (c + 1), 512:2048]
            )
        # gpsimd queue: mask + bulk tail (SWDGE)
        mask_sb = consts.tile([128, KT, NQ], BF, name="mask_sb", tag="mask")
        maskT_r = maskT_d.rearrange("(m p) q -> p m q", p=128)
        nc.gpsimd.dma_start(out=mask_sb[:, 0:4, :], in_=maskT_r[:, 0:4, :])
        for c in range(2):
            nc.gpsimd.dma_start(
                out=xT_sb[c][:, 2048:3072], in_=xT_d[128 * c : 128 * (c + 1), 2048:3072]
            )
        for c in range(2):
            nc.gpsimd.dma_start(
                out=xT_sb[c][:, 3072:4096], in_=xT_d[128 * c : 128 * (c + 1), 3072:4096]
            )
        for ch in range(1, 8):
            nc.gpsimd.dma_start(
                out=mask_sb[:, 4 * ch : 4 * (ch + 1), :],
                in_=maskT_r[:, 4 * ch : 4 * (ch + 1), :],
            )
        wp_sb = consts.tile([128, 4, C], BF, name="wp_sb", tag="wp")
        nc.gpsimd.dma_start(
            out=wp_sb, in_=wproj2_d.rearrange("(g p) c -> p g c", p=128)
        )
        bias_sb = consts.tile([128, 2], F32, name="bias_sb", tag="bias")
        nc.gpsimd.dma_start(out=bias_sb, in_=bias2_d[:])

        # ---------------- persistent SBUF ----------------
        qT_sb = [
            consts.tile([128, NQ], BF, name=f"qT_sb{g}", tag=f"qT{g}") for g in range(2)
        ]
        kT_sb = [
            consts.tile([128, N], BF, name=f"kT_sb{g}", tag=f"kT{g}") for g in range(2)
        ]
        # v tiles with a fused ones column: per ktile, 34-wide blocks
        # [v_h (32) | 1 | pad] so lhsT [128, 33] per head fuses the softmax
        # denominator into the pv matmul as output row 32.
        v_all = consts.tile([128, KT, 34 * H], BF, name="v_all", tag="vall")
        rec_sb = consts.tile([128, 4 * NQ], F32, name="rec_sb", tag="rec")
        bc_cat = consts.tile([128, 4 * NQ], F32, name="bc_cat", tag="bc")
        o_cat = [
            consts.tile([128, NQ], BF, name=f"o_cat{j}", tag=f"oc{j}") for j in range(4)
        ]
        f_acc = consts.tile([128, 1024], F32, name="f_acc", tag="facc")
        nrm_scr = dram_pool.tile([4, 2, NQ], F32)

        nc.gpsimd.memset(bc_cat, 0.0)
        v_r = v_all.rearrange("p m (h w) -> p m h w", h=H)
        for h in range(H):
            nc.gpsimd.memset(v_r[:, :, h, 32:34], 1.0)

        # ---------------- phase-1 emissions ----------------
        def emit_q(g, tag):
            q_ps = ps.tile(
                [128, TAGCOLS[tag]], F32, name="q_ps", tag=tag, bufs=TAGBUFS[tag]
            )
            for c in range(2):
                nc.tensor.matmul(
                    out=q_ps[:, 0:NQ],
                    lhsT=w_sb[c][:, 128 * g : 128 * (g + 1)],
                    rhs=xq_sb[c],
                    start=(c == 0),
                    stop=(c == 1),
                )
            nc.vector.tensor_copy(out=qT_sb[g], in_=q_ps[:, 0:NQ])

        def emit_kT(g, n, tag):
            k_ps = ps.tile(
                [128, TAGCOLS[tag]], F32, name="k_ps", tag=tag, bufs=TAGBUFS[tag]
            )
            for c in range(2):
                nc.tensor.matmul(
                    out=k_ps[:, 0:512],
                    lhsT=w_sb[c][:, 256 + 128 * g : 256 + 128 * (g + 1)],
                    rhs=xT_sb[c][:, 512 * n : 512 * (n + 1)],
                    start=(c == 0),
                    stop=(c == 1),
                )
            nc.vector.tensor_copy(
                out=kT_sb[g][:, 512 * n : 512 * (n + 1)], in_=k_ps[:, 0:512]
            )

        def emit_v(m, tag):
            v_ps = ps.tile(
                [128, TAGCOLS[tag]], F32, name="v_ps", tag=tag, bufs=TAGBUFS[tag]
            )
            for c in range(2):
                nc.tensor.matmul(
                    out=v_ps[:, 0:C],
                    lhsT=xT_sb[c][:, 128 * m : 128 * (m + 1)],
                    rhs=w_sb[c][:, 512:768],
                    start=(c == 0),
                    stop=(c == 1),
                )
            nc.vector.tensor_copy(
                out=v_r[:, m, :, 0:32],
                in_=v_ps[:, 0:C].rearrange("p (h w) -> p h w", h=H),
            )

        # head: q(g0) + v(0..19) + kT(0, 0..3), spread over the psum rings
        head_ems = [lambda t: emit_q(0, t)]
        head_ems += [(lambda m: lambda t: emit_v(m, t))(m) for m in range(20)]
        head_ems += [(lambda n: lambda t: emit_kT(0, n, t))(n) for n in range(4)]
        ring = ["sA", "sB", "pv", "pv"]
        for i, em in enumerate(head_ems):
            em(ring[i % 4])

        # per-pair emission queues (popped one per step, alternating rings)
        emq = {j: [] for j in range(4)}
        emq[0] = [
            (lambda m: lambda t: emit_v(m, t))(20),
            (lambda m: lambda t: emit_v(m, t))(21),
            (lambda n: lambda t: emit_kT(0, n, t))(4),
            (lambda m: lambda t: emit_v(m, t))(22),
            (lambda m: lambda t: emit_v(m, t))(23),
            (lambda n: lambda t: emit_kT(0, n, t))(5),
            (lambda m: lambda t: emit_v(m, t))(24),
            (lambda m: lambda t: emit_v(m, t))(25),
            (lambda n: lambda t: emit_kT(0, n, t))(6),
            (lambda m: lambda t: emit_v(m, t))(26),
            (lambda m: lambda t: emit_v(m, t))(27),
            (lambda n: lambda t: emit_kT(0, n, t))(7),
            (lambda m: lambda t: emit_v(m, t))(28),
            (lambda m: lambda t: emit_v(m, t))(29),
            (lambda m: lambda t: emit_v(m, t))(30),
            (lambda m: lambda t: emit_v(m, t))(31),
        ]
        emq[1] = [(lambda n: lambda t: emit_kT(1, n, t))(n) for n in range(8)]
        emq[1].append(lambda t: emit_q(1, t))

        # ---------------- phase 2: attention ----------------
        pv_tiles = {}

        def get_pv(j):
            if j not in pv_tiles:
                t = ps.tile([128, 512], F32, name="pv_t", tag="pv", bufs=2)
                pv_tiles[j] = t
                if j < 2:
                    # 1.0 keeps reciprocal_approx_fast well-defined on rows
                    # the pv matmuls never write; later pairs reuse the slot,
                    # whose unwritten rows still hold this memset.
                    nc.vector.memset(t, 1.0)
            return pv_tiles[j]

        def issue_pv(j, kind, m, p_t):
            hA, hB = 2 * j, 2 * j + 1
            pv_t = get_pv(j)
            ktcols = (
                [(m, 0), (m + 1, 512)] if kind == "A" else [(m, 0)]
            )
            for h, base in ((hA, 0), (hB, 1)):
                rows = (0, 33) if base == 0 else (64, 97)
                tp = (0, 0) if base == 0 else (0, 64)
                for kt, off in ktcols:
                    cof = off + (0 if base == 0 else (1024 if kind == "A" else 512))
                    nc.tensor.matmul(
                        out=pv_t[rows[0] : rows[1], 0:NQ],
                        lhsT=v_all[:, kt, 34 * h : 34 * h + 33],
                        rhs=p_t[:, cof : cof + NQ],
                        start=(kt == 0),
                        stop=(kt == KT - 1),
                        tile_position=tp,
                        skip_group_check=True,
                    )

        def epilogue(j):
            pv_t = pv_tiles.pop(j)
            nc.vector.reciprocal_approx_fast(
                out=rec_sb[:, NQ * j : NQ * j + NQ], in_=pv_t[:, 0:NQ]
            )
            for half, prow, orow in ((0, 32, 0), (1, 96, 64)):
                nc.sync.dma_start(
                    out=nrm_scr[j, half, :],
                    in_=rec_sb[prow : prow + 1, NQ * j : NQ * j + NQ],
                )
                row = nrm_scr[j, half : half + 1, :]
                bcast = bass.AP(
                    tensor=row.tensor,
                    offset=row.offset,
                    ap=[[0, 32]] + list(row.ap[1:]),
                )
                nc.sync.dma_start(
                    out=bc_cat[orow : orow + 32, NQ * j : NQ * j + NQ],
                    in_=bcast,
                )
            nc.vector.tensor_mul(
                out=o_cat[j],
                in0=pv_t[:, 0:NQ],
                in1=bc_cat[:, NQ * j : NQ * j + NQ],
            )
            f_ps = ps.tile([128, 1024], F32, name="f_ps", tag="sB", bufs=1)
            for t in range(2):
                nc.tensor.matmul(
                    out=f_ps[:, 512 * t : 512 * (t + 1)],
                    lhsT=wp_sb[:, j, 128 * t : 128 * (t + 1)],
                    rhs=o_cat[j],
                    start=True,
                    stop=True,
                )
            if j == 0:
                for t in range(2):
                    nc.vector.tensor_scalar(
                        out=f_acc[:, 512 * t : 512 * (t + 1)],
                        in0=f_ps[:, 512 * t : 512 * (t + 1)],
                        scalar1=bias_sb[:, t : t + 1],
                        scalar2=None,
                        op0=ADD,
                    )
            else:
                nc.vector.tensor_add(out=f_acc, in0=f_acc, in1=f_ps)

        # step list: per pair, (A: kt m,m+1 | B: kt m) pattern, 3 kt / cycle
        all_steps = []
        for j in range(4):
            for cyc in range(10):
                all_steps.append((j, "A", 3 * cyc))
                all_steps.append((j, "B", 3 * cyc + 2))
            all_steps.append((j, "A", 30))

        em_ring = {0: 0, 1: 0, 2: 0, 3: 0}
        # pv matmuls lag a FULL cycle (2 steps): the PE is in-order, so a
        # 1-step lag would stall it behind that step's exp+mask chain.
        pend = []
        for j, kind, m in all_steps:
            hA, hB = 2 * j, 2 * j + 1
            gA = hA // 4
            pA, pB = 32 * (hA % 4), 32 * (hB % 4)
            ncols = 2048 if kind == "A" else 1024
            s_t = ps.tile(
                [128, ncols], F32, name="s_t",
                tag=("sA" if kind == "A" else "sB"), bufs=1,
            )
            p_t = pp.tile(
                [128, ncols], BF, name="p_t",
                tag=("pA" if kind == "A" else "pB"), bufs=3,
            )
            kts = [m, m + 1] if kind == "A" else [m]
            # scores: out (kpos, qrow) per (head, ktile)
            for hi, (h, p_off) in enumerate(((hA, pA), (hB, pB))):
                for ki, kt in enumerate(kts):
                    cof = (hi * len(kts) + ki) * 512
                    nc.tensor.matmul(
                        out=s_t[:, cof : cof + NQ],
                        lhsT=kT_sb[gA][p_off : p_off + 32, 128 * kt : 128 * (kt + 1)],
                        rhs=qT_sb[gA][p_off : p_off + 32, :],
                        start=True,
                        stop=True,
                        tile_position=(p_off, 0),
                    )
            nc.scalar.activation(out=p_t, in_=s_t, func=EXP)
            if kind == "A":
                nc.vector.tensor_mul(
                    out=p_t[:, 0:1024], in0=p_t[:, 0:1024], in1=mask_sb[:, m : m + 2, :]
                )
                nc.vector.tensor_mul(
                    out=p_t[:, 1024:2048],
                    in0=p_t[:, 1024:2048],
                    in1=mask_sb[:, m : m + 2, :],
                )
            else:
                nc.vector.tensor_mul(
                    out=p_t[:, 0:512], in0=p_t[:, 0:512], in1=mask_sb[:, m, :]
                )
                nc.vector.tensor_mul(
                    out=p_t[:, 512:1024], in0=p_t[:, 512:1024], in1=mask_sb[:, m, :]
                )
            if len(pend) == 2:
                pj, pkind, pm, pp_t = pend.pop(0)
                issue_pv(pj, pkind, pm, pp_t)
                if pkind == "A" and pm == 30:
                    epilogue(pj)
            if emq[j]:
                em = emq[j].pop(0)
                tag = ("sB", "sA")[em_ring[j] % 2]
                em_ring[j] += 1
                em(tag)
            pend.append((j, kind, m, p_t))

        for pj, pkind, pm, pp_t in pend:
            issue_pv(pj, pkind, pm, pp_t)
            if pkind == "A" and pm == 30:
                epilogue(pj)

        for t in range(2):
            nc.sync.dma_start(
                out=out_d[128 * t : 128 * (t + 1), :],
                in_=f_acc[:, 512 * t : 512 * (t + 1)],
            )

    nc.compile()
    return nc


def _get_nc():
    if "nc" not in _CACHE:
        _CACHE["nc"] = build_kernel()
    return _CACHE["nc"]


def _prep_inputs(x, adj, w_qkv, w_proj, b_proj):
    x = np.asarray(x, dtype=np.float32).reshape(N, C)
    adj = np.asarray(adj).reshape(N, N)
    w_qkv = np.asarray(w_qkv, dtype=np.float32)
    w_proj = np.asarray(w_proj, dtype=np.float32)
    b_proj = np.asarray(b_proj, dtype=np.float32)

    scale = float(DH) ** -0.5
    wqkvT = w_qkv.T.copy()
    wqkvT[:, 0:C] *= scale  # fold attention scale into q projection
    wqkvT = np.ascontiguousarray(wqkvT, dtype=BF16NP)
    wprojT = w_proj.T.astype(np.float32)  # [C (contraction), C (out)]
    # zero-padded reorder: block j rows 0:32 = head 2j, rows 64:96 =
    # head 2j+1, rest zero (matches pv bank partition layout)
    wproj2 = np.zeros((4 * 128, C), dtype=np.float32)
    for j in range(4):
        wproj2[128 * j + 0 : 128 * j + 32] = wprojT[64 * j : 64 * j + 32]
        wproj2[128 * j + 64 : 128 * j + 96] = wprojT[64 * j + 32 : 64 * j + 64]
    wproj2 = np.ascontiguousarray(wproj2, dtype=BF16NP)
    bias2 = np.ascontiguousarray(b_proj.reshape(2, 128).T, dtype=np.float32)
    xT = np.ascontiguousarray(x.T, dtype=BF16NP)
    adjT = (adj > 0).astype(BF16NP).T  # [kpos, qrow] 0/1

    in_maps = []
    for i in range(NCORES):
        sl = slice(NQ * i, NQ * (i + 1))
        in_maps.append(
            {
                "xT": xT,
                "xqT": np.ascontiguousarray(xT[:, sl]),
                "wqkv": wqkvT,
                "wproj2": wproj2,
                "bias2": bias2,
                "maskT": np.ascontiguousarray(adjT[:, sl]),
            }
        )
    return in_maps


def run_on_hw(inputs, trace=False):
    from concourse.bass_utils import run_bass_kernel_spmd

    if trace:
        import axon_profile_shim  # noqa: F401

    nc = _get_nc()
    in_maps = _prep_inputs(**inputs)
    res = run_bass_kernel_spmd(
        nc, in_maps, core_ids=list(range(NCORES)), trace=trace
    )
    out = np.empty((1, N, C), dtype=np.float32)
    for i in range(NCORES):
        out[0, NQ * i : NQ * (i + 1), :] = res.results[i]["out"].T
    return out, res


def kernel(x, adj, w_qkv, w_proj, b_proj):
    out, _ = run_on_hw(
        {"x": x, "adj": adj, "w_qkv": w_qkv, "w_proj": w_proj, "b_proj": b_proj}
    )
    return out
